# revision 1
# baseline (speedup 1.0000x reference)
"""Trainium2 Bass kernel for a 2-layer GCN (AttributeDecoder):

    out = relu(adj @ relu(adj @ (X @ W1)) @ W2)

with N=8192, D_IN=64, D_HID=128, D_OUT=256, fp32 in/out.

Strategy (8 NeuronCores, SPMD):
  - Row-shard adj across cores: core i owns rows [1024*i, 1024*(i+1)).
  - adj is down-converted to bf16 ON HOST and packed per core into 16
    contiguous "quad" slabs [128, 4096]: slab q holds k-blocks 4q..4q+3 of
    adjT_i (k on the partition axis so the PE reduces over partitions) as
    one linear 1MB HBM block -> each slab loads with a single max-rate DMA.
  - The whole bf16 shard (16MB) fits in SBUF, so it is read from HBM
    exactly ONCE: layer 1 streams the 16 quads as they arrive; layer 2
    reuses them from SBUF with zero adj HBM traffic (target_regime=memory:
    per-core HBM traffic drops from 55MB (f32 + partial re-read) to ~20MB).
  - bf16 also doubles the PE rate (78.6 TF/s vs 39.3 f32r) and allows
    1024-wide moving operands: each layer is just 64 matmuls accumulating
    into one [128, 1024] PSUM tile.
  - X, W1, W2 replicated (bf16). On-chip per core:
      XW1 = X @ W1                      ([8192, 128] node-major, bf16)
      H1^T_own = relu(adj_i @ XW1)^T    64 matmuls, psum[h, m] (+ relu)
      PE-transpose -> H1_own, AllGather (bf16, 256KB/rank -> 2MB)
      AH^T = (adj_i @ H1)^T             64 matmuls from SBUF-resident quads
      OUT^T_own = relu(W2^T @ AH^T)     2 matmuls + relu, f32 out
    Layer 2 uses the associativity flip (adj@H1)@W2 instead of adj@(H1@W2):
    the streaming contraction is against H1's 128 columns, not G's 256.
  - Host gathers outT_i ([256, 1024] f32) from each core and transposes.

Accuracy: bf16 rounding of adj/X/H1 gives ~1e-3 end-to-end max-rel error
(gate is 2e-2); accumulation stays fp32 in PSUM throughout.
"""

import numpy as np

N = 8192
D_IN, D_HID, D_OUT = 64, 128, 256
NCORES = 8
SHARD = N // NCORES  # 1024
KB = N // 128  # 64 k-blocks of 128
QUADS = KB // 4  # 16 quad slabs, 4 k-blocks each


def _build_nc(reps: int = 1, l1_only: bool = False, no_coll: bool = False,
              no_adj: bool = False, no_tr: bool = False, no_l2mm: bool = False,
              h1c_hw: bool = False, no_tr_no_out: bool = False,
              no_tr_half: bool = False):
    from concourse import bacc
    import concourse.mybir as mybir
    import concourse.tile as tile
    from concourse.bass import ts
    from concourse.masks import make_identity

    f32 = mybir.dt.float32
    f32r = mybir.dt.float32r
    bf16 = mybir.dt.bfloat16
    Relu = mybir.ActivationFunctionType.Relu

    nc = bacc.Bacc("TRN2", target_bir_lowering=False, debug=False, num_devices=NCORES)

    adjP = nc.dram_tensor("adjP", [QUADS * 128, 4 * SHARD], bf16,
                          kind="ExternalInput").ap()
    xT = nc.dram_tensor("xT", [D_IN, N], bf16, kind="ExternalInput").ap()
    w1 = nc.dram_tensor("w1", [D_IN, D_HID], bf16, kind="ExternalInput").ap()
    w2 = nc.dram_tensor("w2", [D_HID, D_OUT], bf16, kind="ExternalInput").ap()
    outT = nc.dram_tensor("outT", [D_OUT, SHARD], f32, kind="ExternalOutput").ap()

    def body(tc, rep):
        nc = tc.nc

        with (
            tc.tile_pool(name="const", bufs=1) as const_pool,
            tc.tile_pool(name="cache", bufs=1) as cache_pool,
            tc.tile_pool(name="h1p", bufs=1) as h1_pool,
            tc.tile_pool(name="copies", bufs=2) as copy_pool,
            tc.tile_pool(name="dram", bufs=1, space="DRAM") as dram_pool,
        ):
            # ---- constants ride the scalar HWDGE ring, ahead of its quads
            xT_sb = const_pool.tile([D_IN, N], bf16, name=f"xTsb{rep}")
            nc.scalar.dma_start(xT_sb[:], xT[:])
            w1_sb = const_pool.tile([D_IN, D_HID], bf16, name=f"w1sb{rep}")
            nc.scalar.dma_start(w1_sb[:], w1[:])
            w2_sb = const_pool.tile([D_HID, D_OUT], bf16, name=f"w2sb{rep}")
            nc.scalar.dma_start(w2_sb[:], w2[:])

            # ---- adj quad slabs: one contiguous 1MB DMA each, alternating
            # between the two HWDGE rings; all 16 stay SBUF-resident.
            quads = []
            for q in range(QUADS):
                qt = cache_pool.tile([128, 4 * SHARD], bf16,
                                     name=f"quad{rep}_{q}", tag=f"quad{q}")
                eng = nc.sync if q % 2 == 0 else nc.scalar
                if not no_adj:
                    eng.dma_start(qt[:], adjP[ts(q, 128), :])
                quads.append(qt)

            # ---- XW1 = X @ W1, node-major bf16 [128, KB*128]
            xw1_all = const_pool.tile([128, N], bf16, name=f"xw1{rep}")
            with tc.tile_pool(name="xw1_ps", bufs=2, space="PSUM") as xw1_ps_pool:
                for j in range(KB):
                    ps = xw1_ps_pool.tile([128, D_HID], f32,
                                          name=f"xw1ps{rep}_{j}", tag="xw1ps")
                    nc.tensor.matmul(ps[:], xT_sb[:, ts(j, 128)], w1_sb[:],
                                     start=True, stop=True)
                    nc.vector.tensor_copy(xw1_all[:, ts(j, 128)], ps[:])

            # ---- layer 1: psum_h[h, m] = sum_k XW1[k, h] * adjT[k, m]
            # h (the PSUM bank) is the OUTER loop: 64 back-to-back matmuls
            # into one bank, then one switch. Alternating banks per matmul
            # breaks PE back-to-back pipelining (each MM pays the full
            # ~(219+N)-cycle fill+drain instead of N). Pass h=1 re-reads the
            # quads from SBUF, so DMA pacing only gates pass h=0.
            h1r = const_pool.tile([D_HID, SHARD], f32r, name=f"h1r{rep}")
            with tc.tile_pool(name="l1_ps", bufs=1, space="PSUM") as l1_ps_pool:
                psum_h = l1_ps_pool.tile([D_HID, SHARD], f32, name=f"l1ps{rep}")
                for h in range(SHARD // 512):
                    for q in range(QUADS):
                        for b in range(4):
                            j = 4 * q + b
                            nc.tensor.matmul(
                                psum_h[:, ts(h, 512)],
                                xw1_all[:, ts(j, 128)],
                                quads[q][:, ts(2 * b + h, 512)],
                                start=(j == 0),
                                stop=(j == KB - 1),
                            )
                    nc.scalar.activation(h1r[:, ts(h, 512)],
                                         psum_h[:, ts(h, 512)], Relu)

            if l1_only:
                nc.sync.dma_start(outT[ts(0, 128), :], h1r[:].bitcast(f32))
                return

            if no_tr or no_tr_no_out or no_tr_half:
                # skip transpose/AllGather/h1c entirely; L2 lhsT = xw1 slices
                # (garbage numerics, isolates the L1+L2+out matmul pipeline)
                nmm = KB // 2 if no_tr_half else KB
                with tc.tile_pool(name="l2_ps", bufs=1, space="PSUM") as l2_ps_pool:
                    psum_ah = l2_ps_pool.tile([D_HID, SHARD], f32,
                                              name=f"l2ps{rep}")
                    for h in range(SHARD // 512):
                        for j in range(nmm):
                            q, c = divmod(j, 4)
                            nc.tensor.matmul(
                                psum_ah[:, ts(h, 512)],
                                xw1_all[:, ts(j, 128)],
                                quads[q][:, ts(2 * c + h, 512)],
                                start=(j == 0),
                                stop=(j == nmm - 1),
                            )
                    ah_sb = copy_pool.tile([D_HID, SHARD], bf16,
                                           name=f"ahsb{rep}", tag="ahsb", bufs=1)
                    nc.vector.tensor_copy(ah_sb[:], psum_ah[:])
                if no_tr_no_out:
                    nc.scalar.dma_start(outT[ts(0, 128), 0:512],
                                        ah_sb[:].bitcast(f32))
                    return
                with tc.tile_pool(name="of_ps", bufs=2, space="PSUM") as of_ps_pool:
                    for ch in range(D_OUT // 128):
                        psum_of = of_ps_pool.tile([128, SHARD], f32,
                                                  name=f"ofps{rep}_{ch}",
                                                  tag="ofps")
                        for h in range(SHARD // 512):
                            nc.tensor.matmul(psum_of[:, ts(h, 512)],
                                             w2_sb[:, ts(ch, 128)],
                                             ah_sb[:, ts(h, 512)],
                                             start=True, stop=True)
                        o_sb = copy_pool.tile([128, SHARD], f32,
                                              name=f"osb{rep}_{ch}", tag="osb",
                                              bufs=2)
                        nc.scalar.activation(o_sb[:], psum_of[:], Relu)
                        nc.scalar.dma_start(outT[ts(ch, 128), :], o_sb[:])
                return

            # ---- PE-transpose H1^T_own into node-major blocks: h1t_all
            # [128, SHARD] where h1t_all[kk, 128*jl + h] = H1_own[128*jl+kk, h].
            # The DRAM copy keeps this EXACT linear layout ([128, SHARD] rows)
            # so both the write and the post-AllGather readbacks move 2KB
            # contiguous lines per partition (256B node-rows would be
            # descriptor-bound at ~10x the cost). After the rank-major
            # AllGather, rank r's rows [128r:128r+128] slice directly into
            # standard k-block lhsT tiles: h1c[:, ts(jl,128)] = block 8r+jl.
            ident_f32 = const_pool.tile([128, 128], f32, name=f"identf{rep}")
            make_identity(nc, ident_f32[:])
            identity = const_pool.tile([128, 128], f32r, name=f"ident{rep}")
            nc.vector.tensor_copy(identity[:], ident_f32[:])
            h1_own_dram = dram_pool.tile([128, SHARD], bf16, name=f"h1own{rep}")
            h1_all_dram = dram_pool.tile([NCORES * 128, SHARD], bf16,
                                         addr_space="Shared", name=f"h1all{rep}")
            h1t_all = copy_pool.tile([128, SHARD], bf16, name=f"h1t{rep}",
                                     tag="h1t", bufs=1)
            with tc.tile_pool(name="tr_ps", bufs=2, space="PSUM") as tr_ps_pool:
                for jl in range(SHARD // 128):
                    tps = tr_ps_pool.tile([128, D_HID], f32r,
                                          name=f"tps{rep}_{jl}", tag="tps")
                    nc.tensor.transpose(tps[:], h1r[:, ts(jl, 128)], identity[:])
                    nc.vector.tensor_copy(h1t_all[:, ts(jl, 128)], tps[:])
            nc.scalar.dma_start(h1_own_dram[:, :], h1t_all[:])

            if not no_coll:
                nc.gpsimd.collective_compute(
                    "AllGather",
                    mybir.AluOpType.bypass,
                    replica_groups=[list(range(NCORES))],
                    ins=[h1_own_dram.opt()],
                    outs=[h1_all_dram.opt()],
                )

            # ---- layer 2: psum_ah[h, m] += H1[k, h] * adjT[k, m], quads
            # straight from SBUF (no adj HBM traffic). Bank-outer loop as in
            # layer 1; all 8 gathered-H1 chunks stay resident for pass h=1.
            with tc.tile_pool(name="l2_ps", bufs=1, space="PSUM") as l2_ps_pool:
                psum_ah = l2_ps_pool.tile([D_HID, SHARD], f32, name=f"l2ps{rep}")
                ah_sb = copy_pool.tile([D_HID, SHARD], bf16, name=f"ahsb{rep}",
                                       tag="ahsb", bufs=1)
                h1cs = []
                for c8 in range(KB // 8):  # 8 gathered-H1 chunks of 8 k-blocks
                    h1c = h1_pool.tile([128, 8 * D_HID], bf16,
                                       name=f"h1c{rep}_{c8}", tag=f"h1c{c8}")
                    # scalar HWDGE: these wait on the AllGather; keeping them
                    # off the sync ring leaves one ring free for the next
                    # rep's quad stream (gpsimd SWDGE measured far slower).
                    if no_coll:
                        nc.scalar.dma_start(h1c[:], h1_own_dram[:, :])
                    else:
                        nc.scalar.dma_start(h1c[:], h1_all_dram[ts(c8, 128), :])
                    h1cs.append(h1c)
                if no_l2mm:
                    h1cs = []
                for h in range(SHARD // 512):
                    for c8 in range(len(h1cs)):
                        for b8 in range(8):
                            j = 8 * c8 + b8
                            q, c = divmod(j, 4)
                            nc.tensor.matmul(
                                psum_ah[:, ts(h, 512)],
                                h1cs[c8][:, ts(b8, 128)],
                                quads[q][:, ts(2 * c + h, 512)],
                                start=(j == 0),
                                stop=(j == KB - 1),
                            )
                    if h1cs:
                        # per-half copy overlaps the PE's other-bank pass
                        nc.vector.tensor_copy(ah_sb[:, ts(h, 512)],
                                              psum_ah[:, ts(h, 512)])
                if no_l2mm:
                    nc.scalar.dma_start(outT[ts(0, 128), 0:512],
                                        h1t_all[:].bitcast(f32))
                    return

            # ---- OUT^T = relu(W2^T @ AH^T)
            with tc.tile_pool(name="of_ps", bufs=2, space="PSUM") as of_ps_pool:
                for ch in range(D_OUT // 128):
                    psum_of = of_ps_pool.tile([128, SHARD], f32,
                                              name=f"ofps{rep}_{ch}", tag="ofps")
                    for h in range(SHARD // 512):
                        nc.tensor.matmul(psum_of[:, ts(h, 512)],
                                         w2_sb[:, ts(ch, 128)],
                                         ah_sb[:, ts(h, 512)],
                                         start=True, stop=True)
                    o_sb = copy_pool.tile([128, SHARD], f32,
                                          name=f"osb{rep}_{ch}", tag="osb", bufs=2)
                    nc.scalar.activation(o_sb[:], psum_of[:], Relu)
                    # scalar ring right after the same-engine relu: zero-wait
                    # at the ring head, so it can't stall the quad stream.
                    nc.scalar.dma_start(outT[ts(ch, 128), :], o_sb[:])

    with tile.TileContext(nc) as tc:
        for rep in range(reps):
            body(tc, rep)
    nc.compile()
    return nc


_NC_CACHE = {}


def get_nc(reps: int = 1, **opts):
    key = (reps, tuple(sorted(opts.items())))
    if key not in _NC_CACHE:
        _NC_CACHE[key] = _build_nc(reps, **opts)
    return _NC_CACHE[key]


def make_in_maps(adj_matrix, node_embs, W1, W2):
    import ml_dtypes

    bf16 = ml_dtypes.bfloat16
    adj_matrix = np.asarray(adj_matrix, dtype=np.float32)
    xT = np.asarray(node_embs, dtype=np.float32).T.astype(bf16)
    w1 = np.asarray(W1, dtype=np.float32).astype(bf16)
    w2 = np.asarray(W2, dtype=np.float32).astype(bf16)
    in_maps = []
    for i in range(NCORES):
        # adjT_i[k, m] = adj[i*SHARD + m, k]; quad q packs k-blocks 4q..4q+3
        # as adjP[q*128 + kk, b*SHARD + m] = adjT_i[(4q+b)*128 + kk, m]
        adjT_i = adj_matrix[i * SHARD:(i + 1) * SHARD, :].T
        adjP = (adjT_i.reshape(QUADS, 4, 128, SHARD)
                .transpose(0, 2, 1, 3)
                .astype(bf16)
                .reshape(QUADS * 128, 4 * SHARD))
        in_maps.append({"adjP": adjP, "xT": xT, "w1": w1, "w2": w2})
    return in_maps


def kernel(adj_matrix, node_embs, W1, W2):
    import concourse.bass_utils as bass_utils

    nc = get_nc(reps=1)
    in_maps = make_in_maps(adj_matrix, node_embs, W1, W2)
    res = bass_utils.run_bass_kernel_spmd(nc, in_maps, core_ids=list(range(NCORES)))
    out = np.concatenate([r["outT"].T for r in res.results], axis=0)
    return np.ascontiguousarray(out, dtype=np.float32)


if __name__ == "__main__":
    rng = np.random.default_rng(0)
    adj = rng.random((N, N), dtype=np.float32)
    x = rng.standard_normal((N, D_IN)).astype(np.float32)
    W1 = (rng.standard_normal((D_IN, D_HID)) / np.sqrt(D_IN)).astype(np.float32)
    W2 = (rng.standard_normal((D_HID, D_OUT)) / np.sqrt(D_HID)).astype(np.float32)
    out = kernel(adj_matrix=adj, node_embs=x, W1=W1, W2=W2)
    h = np.maximum(adj @ (x @ W1), 0)
    expected = np.maximum(adj @ (h @ W2), 0)
    err = np.abs(out - expected).max() / np.abs(expected).max()
    print("rel err vs numpy:", err)



# revision 10
# speedup vs baseline: 2.5452x; 2.5452x over previous
"""Trainium2 Bass kernel for a 2-layer GCN (AttributeDecoder):

    out = relu(adj @ relu(adj @ (X @ W1)) @ W2)

with N=8192, D_IN=64, D_HID=128, D_OUT=256, fp32 in/out.

Strategy (8 NeuronCores, SPMD), v2:
  - Row-shard adj across cores: core i owns rows [1024*i, 1024*(i+1)).
  - adj is down-converted to fp8-e4m3 ON HOST (adj entries are U[0,1); the
    propagation sums are sign-coherent after relu, so fp8 adj costs only
    ~3e-3 end-to-end rel err vs the 2e-2 gate). The PE accepts a mixed
    bf16-stationary x fp8-moving matmul (HW-verified exact), so XW1/H1
    stay bf16. adj HBM traffic halves vs bf16: 8MB/core, read ONCE.
  - Slab packing is m-half-major: slab s in [0,8) holds k-blocks 8s..8s+7
    for own-node half m in [0,512); slabs 8..15 the same k-blocks for
    m in [512,1024). Each L1 PSUM-bank pass consumes a contiguous slab
    stream with ZERO bank alternation, and H1's first half is finished
    (and AllGather'd) while the second half still streams.
  - Layer 1: psum_h[h, m] = sum_k XW1[k, h] * adjT[k, m]; 64 back-to-back
    MMs per half. relu -> bf16, per-128-block xbar-DMA transpose (PE never
    transposes), DMA to DRAM, per-half AllGather (128KB/rank) overlapped
    under the other half's compute / L2 head.
  - Layer 2 re-reads the SBUF-resident slabs (zero adj HBM traffic);
    lhsT k-blocks ordered by availability: own shard straight from SBUF
    (no readback), then AG#0 ranks, then AG#1 ranks.
  - OUT^T = relu(W2^T @ AH^T): 4 MMs + relu, f32 out.
  - XW1 = X @ W1 is row-tiled (K=64 pairs at tile_position (0,0)/(64,0))
    ahead of L1; host packs xT accordingly.
  - Host gathers outT_i ([256, 1024] f32) from each core and transposes.
"""

import numpy as np

N = 8192
D_IN, D_HID, D_OUT = 64, 128, 256
NCORES = 8
SHARD = N // NCORES  # 1024
KB = N // 128  # 64 k-blocks of 128
SLABS = 16  # slab s: k-blocks 8*(s%8)..8*(s%8)+7, m-half s//8


def _build_nc(reps: int = 1, no_adj: bool = False, l1_only: bool = False,
              no_coll: bool = False, no_l2: bool = False, no_out: bool = False):
    from concourse import bacc
    import concourse.mybir as mybir
    import concourse.tile as tile
    from concourse.bass import ts
    from concourse.masks import make_identity

    f32 = mybir.dt.float32
    f32r = mybir.dt.float32r
    bf16 = mybir.dt.bfloat16
    f8 = mybir.dt.float8e4
    Relu = mybir.ActivationFunctionType.Relu

    nc = bacc.Bacc("TRN2", target_bir_lowering=False, debug=False, num_devices=NCORES)

    adjS = nc.dram_tensor("adjS", [SLABS * 128, 8 * 512], f8,
                          kind="ExternalInput").ap()
    xTd = nc.dram_tensor("xTd", [128, N // 2], bf16, kind="ExternalInput").ap()
    w1d = nc.dram_tensor("w1d", [128, D_HID], bf16, kind="ExternalInput").ap()
    w2 = nc.dram_tensor("w2", [D_HID, D_OUT], bf16, kind="ExternalInput").ap()
    outT = nc.dram_tensor("outT", [D_OUT, SHARD], f32, kind="ExternalOutput").ap()

    def body(tc, rep):
        nc = tc.nc

        with (
            tc.tile_pool(name="const", bufs=1) as const_pool,
            tc.tile_pool(name="cache", bufs=1) as cache_pool,
            tc.tile_pool(name="h1p", bufs=1) as h1_pool,
            tc.tile_pool(name="copies", bufs=2) as copy_pool,
            tc.tile_pool(name="dram", bufs=1, space="DRAM") as dram_pool,
        ):
            # ---- ring split: the SYNC ring carries ONLY the slab stream
            # (one HWDGE ring fans out to all 16 SDMA engines, so a single
            # ring sustains full HBM rate); the SCALAR ring carries
            # constants first and is then FREE for the mid-stream H1
            # transpose/write/readback chain — HWDGE rings drain FIFO, so
            # putting that chain on the slab ring would queue it behind
            # ~10us of remaining slabs.
            xTd_sb = const_pool.tile([128, N // 2], bf16, name=f"xTsb{rep}")
            for c in range(4):
                nc.scalar.dma_start(xTd_sb[:, ts(c, 1024)], xTd[:, ts(c, 1024)])
            w1_sb = const_pool.tile([128, D_HID], bf16, name=f"w1sb{rep}")
            nc.scalar.dma_start(w1_sb[:], w1d[:])
            w2_sb = const_pool.tile([D_HID, D_OUT], bf16, name=f"w2sb{rep}")
            nc.scalar.dma_start(w2_sb[:], w2[:])

            # ---- adj slabs: one contiguous 512KB DMA each, all on the
            # sync ring; all 16 stay SBUF-resident for layer 2's re-read.
            slabs = []
            for s in range(SLABS):
                st = cache_pool.tile([128, 8 * 512], f8,
                                     name=f"slab{rep}_{s}", tag=f"slab{s}")
                if not no_adj:
                    nc.sync.dma_start(st[:], adjS[ts(s, 128), :])
                slabs.append(st)

            # ---- XW1 = X @ W1, node-major bf16; K=64 pairs run
            # concurrently in row-groups (0,0)/(64,0). The two row-group
            # MMs MUST land in different PSUM banks (same-bank row-split
            # pairs hang the device), so even node-blocks accumulate in
            # psA / odd in psB, and xw1_all is laid out
            # [even blocks 0..4095 | odd blocks 4096..8191].
            xw1_all = const_pool.tile([128, N], bf16, name=f"xw1{rep}")

            def xw1_col(j):  # column of node-block j's lhsT in xw1_all
                return (j // 2) * 128 + (N // 2 if j % 2 else 0)

            with tc.tile_pool(name="xw1_ps", bufs=2, space="PSUM") as xw1_ps_pool:
                for g in range(8):  # group g = pairs 4g..4g+3
                    psA = xw1_ps_pool.tile([128, 512], f32,
                                           name=f"xw1psA{rep}_{g}", tag="xw1psA")
                    psB = xw1_ps_pool.tile([128, 512], f32,
                                           name=f"xw1psB{rep}_{g}", tag="xw1psB")
                    for v in range(4):  # pair u computes blocks (2u, 2u+1)
                        u = 4 * g + v
                        nc.tensor.matmul(psA[:, ts(v, 128)],
                                         xTd_sb[0:64, ts(u, 128)],
                                         w1_sb[0:64, :], start=True, stop=True)
                        nc.tensor.matmul(psB[:, ts(v, 128)],
                                         xTd_sb[64:128, ts(u, 128)],
                                         w1_sb[64:128, :], start=True, stop=True)
                    nc.vector.tensor_copy(xw1_all[:, ts(g, 512)], psA[:])
                    nc.vector.tensor_copy(xw1_all[:, N // 2 + 512 * g:
                                                  N // 2 + 512 * (g + 1)], psB[:])

            # ---- identity for PE-mode transpose
            ident_f32 = const_pool.tile([128, 128], f32, name=f"identf{rep}")
            make_identity(nc, ident_f32[:])
            identity = const_pool.tile([128, 128], f32r, name=f"ident{rep}")
            nc.vector.tensor_copy(identity[:], ident_f32[:])

            # ---- layer 1 + per-half transpose/AllGather/readback.
            # h1c[(half, r)] [128, 512] = rank r's 4 k-blocks 8r+4*half+blk
            h1t = []      # own H1^T halves, node-major [128, 512]: cols blk*128+h
            h1c = {}
            with tc.tile_pool(name="l1_ps", bufs=1, space="PSUM") as l1_ps_pool:
                psum_h = l1_ps_pool.tile([D_HID, SHARD], f32, name=f"l1ps{rep}")
                for half in range(2):
                    for j in range(KB):
                        s = 8 * half + j // 8
                        u = j % 8
                        nc.tensor.matmul(
                            psum_h[:, ts(half, 512)],
                            xw1_all[:, xw1_col(j):xw1_col(j) + 128],
                            slabs[s][:, ts(u, 512)],
                            start=(j == 0),
                            stop=(j == KB - 1),
                        )
                    h1r = copy_pool.tile([D_HID, 512], f32r,
                                         name=f"h1r{rep}_{half}", tag="h1r")
                    nc.scalar.activation(h1r[:], psum_h[:, ts(half, 512)], Relu)
                    ht = h1_pool.tile([128, 512], bf16, name=f"h1t{rep}_{half}",
                                      tag=f"h1t{half}")
                    with tc.tile_pool(name=f"tr_ps{half}", bufs=2,
                                      space="PSUM") as tr_ps_pool:
                        for blk in range(4):
                            tps = tr_ps_pool.tile([128, D_HID], f32r,
                                                  name=f"tps{rep}_{half}_{blk}",
                                                  tag="tps")
                            nc.tensor.transpose(tps[:], h1r[:, ts(blk, 128)],
                                                identity[:])
                            nc.vector.tensor_copy(ht[:, ts(blk, 128)], tps[:])
                    h1t.append(ht)
                    h1_own_dram = dram_pool.tile([128, 512], bf16,
                                                 name=f"h1own{rep}_{half}")
                    nc.scalar.dma_start(h1_own_dram[:, :], ht[:])
                    h1_all_dram = dram_pool.tile([NCORES * 128, 512], bf16,
                                                 addr_space="Shared",
                                                 name=f"h1all{rep}_{half}")
                    if not no_coll:
                        nc.gpsimd.collective_compute(
                            "AllGather",
                            mybir.AluOpType.bypass,
                            replica_groups=[list(range(NCORES))],
                            ins=[h1_own_dram.opt()],
                            outs=[h1_all_dram.opt()],
                        )
                    if not l1_only:
                        for r in range(NCORES):
                            t = h1_pool.tile([128, 512], bf16,
                                             name=f"h1c{rep}_{half}_{r}",
                                             tag=f"h1c{half}_{r}")
                            if no_coll:
                                nc.scalar.dma_start(t[:], h1_own_dram[:, :])
                            else:
                                nc.scalar.dma_start(t[:],
                                                    h1_all_dram[ts(r, 128), :])
                            h1c[(half, r)] = t

            if l1_only:
                nc.sync.dma_start(outT[ts(0, 128), 0:256], h1t[0][:].bitcast(f32))
                return

            def l2_lhsT(kb):
                # lhsT slice for global k-block kb (from the gathered H1)
                r, jl = divmod(kb, 8)
                half_src, blk = divmod(jl, 4)
                return h1c[(half_src, r)][:, ts(blk, 128)]

            # ---- layer 2: psum_ah[h, m] += H1[k, h] * adjT[k, m]; slabs
            # straight from SBUF. k-blocks ordered by availability:
            # AG#0 ranks (jl 0..3), then AG#1 ranks (jl 4..7).
            korder = [8 * r + jl for jl in range(8) for r in range(NCORES)]
            with tc.tile_pool(name="l2_ps", bufs=1, space="PSUM") as l2_ps_pool:
                psum_ah = l2_ps_pool.tile([D_HID, SHARD], f32, name=f"l2ps{rep}")
                ah_sb = copy_pool.tile([D_HID, SHARD], bf16, name=f"ahsb{rep}",
                                       tag="ahsb", bufs=1)
                for half in range(2):
                    if not no_l2:
                        for idx, kb in enumerate(korder):
                            s = 8 * half + kb // 8
                            u = kb % 8
                            nc.tensor.matmul(
                                psum_ah[:, ts(half, 512)],
                                l2_lhsT(kb),
                                slabs[s][:, ts(u, 512)],
                                start=(idx == 0),
                                stop=(idx == KB - 1),
                            )
                        nc.vector.tensor_copy(ah_sb[:, ts(half, 512)],
                                              psum_ah[:, ts(half, 512)])
            if no_l2 or no_out:
                nc.sync.dma_start(outT[ts(0, 128), 0:256], h1t[0][:].bitcast(f32))
                return

            # ---- OUT^T = relu(W2^T @ AH^T)
            with tc.tile_pool(name="of_ps", bufs=2, space="PSUM") as of_ps_pool:
                for ch in range(D_OUT // 128):
                    psum_of = of_ps_pool.tile([128, SHARD], f32,
                                              name=f"ofps{rep}_{ch}", tag="ofps")
                    for hm in range(2):
                        nc.tensor.matmul(psum_of[:, ts(hm, 512)],
                                         w2_sb[:, ts(ch, 128)],
                                         ah_sb[:, ts(hm, 512)],
                                         start=True, stop=True)
                    o_sb = copy_pool.tile([128, SHARD], f32,
                                          name=f"osb{rep}_{ch}", tag="osb", bufs=2)
                    nc.scalar.activation(o_sb[:], psum_of[:], Relu)
                    nc.sync.dma_start(outT[ts(ch, 128), :], o_sb[:])

    with tile.TileContext(nc) as tc:
        for rep in range(reps):
            body(tc, rep)
    nc.compile()
    return nc


_NC_CACHE = {}


def get_nc(reps: int = 1, **opts):
    key = (reps, tuple(sorted(opts.items())))
    if key not in _NC_CACHE:
        _NC_CACHE[key] = _build_nc(reps, **opts)
    return _NC_CACHE[key]


def make_in_maps(adj_matrix, node_embs, W1, W2):
    import ml_dtypes

    bf16 = ml_dtypes.bfloat16
    f8 = ml_dtypes.float8_e4m3
    adj_matrix = np.asarray(adj_matrix, dtype=np.float32)
    xT = np.asarray(node_embs, dtype=np.float32).T  # [64, 8192]
    # row-tile pair packing: xTd[0:64, u*128+c] = xT[:, (2u)*128+c],
    #                        xTd[64:128, u*128+c] = xT[:, (2u+1)*128+c]
    xr = xT.reshape(D_IN, N // 256, 2, 128)
    xTd = np.concatenate(
        [xr[:, :, 0, :].reshape(D_IN, N // 2), xr[:, :, 1, :].reshape(D_IN, N // 2)],
        axis=0).astype(bf16)
    w1d = np.concatenate([np.asarray(W1, np.float32)] * 2, axis=0).astype(bf16)
    w2 = np.asarray(W2, dtype=np.float32).astype(bf16)
    in_maps = []
    for i in range(NCORES):
        # adjT_i[k, m] = adj[i*SHARD + m, k]; slab (half*8+s) packs
        # adjS[(half*8+s)*128 + kk, u*512 + m] = adjT_i[(8s+u)*128+kk, half*512+m]
        adjT_i = adj_matrix[i * SHARD:(i + 1) * SHARD, :].T
        adjS = (adjT_i.reshape(8, 8, 128, 2, 512)
                .transpose(3, 0, 2, 1, 4)
                .astype(f8)
                .reshape(SLABS * 128, 8 * 512))
        in_maps.append({"adjS": adjS, "xTd": xTd, "w1d": w1d, "w2": w2})
    return in_maps


def kernel(adj_matrix, node_embs, W1, W2):
    import concourse.bass_utils as bass_utils

    nc = get_nc(reps=1)
    in_maps = make_in_maps(adj_matrix, node_embs, W1, W2)
    res = bass_utils.run_bass_kernel_spmd(nc, in_maps, core_ids=list(range(NCORES)))
    out = np.concatenate([r["outT"].T for r in res.results], axis=0)
    return np.ascontiguousarray(out, dtype=np.float32)


if __name__ == "__main__":
    rng = np.random.default_rng(0)
    adj = rng.random((N, N), dtype=np.float32)
    x = rng.standard_normal((N, D_IN)).astype(np.float32)
    W1 = (rng.standard_normal((D_IN, D_HID)) / np.sqrt(D_IN)).astype(np.float32)
    W2 = (rng.standard_normal((D_HID, D_OUT)) / np.sqrt(D_HID)).astype(np.float32)
    out = kernel(adj_matrix=adj, node_embs=x, W1=W1, W2=W2)
    h = np.maximum(adj @ (x @ W1), 0)
    expected = np.maximum(adj @ (h @ W2), 0)
    err = np.abs(out - expected).max() / np.abs(expected).max()
    print("rel err vs numpy:", err)
